# revision 1
# baseline (speedup 1.0000x reference)
"""GroupedQueryAttention kernel for 8 TRN2 NeuronCores.

Sharding: the 8 (batch, kv-group) pairs map 1:1 onto the 8 cores
(B=2 x G=4). Each core holds its group's K/V projection rows, the
matching 4-query-head slice of Wq, and the matching 256-column slice
of Wo (row-sharded out_proj). Each core produces a partial
[Q, DIM] out-proj contribution; the 4-way group reduction + bias is
done on host (cheap: 2*4*2048*1024 adds).
"""
import numpy as np
import jax
import jax.numpy as jnp
from functools import partial

DIM = 1024
NUM_HEADS = 16
NUM_GROUPS = 4
HEAD_DIM = DIM // NUM_HEADS          # 64
HPG = NUM_HEADS // NUM_GROUPS        # 4
GQ = HPG * HEAD_DIM                  # 256: query-proj rows per group
B = 2
SCALE = 1.0 / np.sqrt(HEAD_DIM)
LN_EPS = 1e-5
NDEV = 8


def _ln(x, w, b):
    m = jnp.mean(x, axis=-1, keepdims=True)
    v = jnp.mean(jnp.square(x - m), axis=-1, keepdims=True)
    return (x - m) * jax.lax.rsqrt(v + LN_EPS) * w + b


@partial(jax.pmap, axis_name="x")
def _gqa_shard(q_in, k_in, v_in, mask, Wq_g, bq_g, Wk_g, bk_g, Wv_g, bv_g,
               qn_w, qn_b, kn_w, kn_b, WoT_g):
    # q_in/k_in/v_in: [Q, DIM]; mask: [Q, KV] bool
    q = q_in @ Wq_g.T + bq_g                         # [Q, 256]
    k = k_in @ Wk_g.T + bk_g                         # [KV, 64]
    v = v_in @ Wv_g.T + bv_g                         # [KV, 64]
    Q = q.shape[0]
    q = q.reshape(Q, HPG, HEAD_DIM).transpose(1, 0, 2)   # [hpg, Q, hd]
    q = _ln(q, qn_w, qn_b)
    k = _ln(k, kn_w, kn_b)                               # [KV, hd]
    scores = jnp.einsum("hqd,kd->hqk", q, k) * SCALE     # [hpg, Q, KV]
    neg = jnp.asarray(jnp.finfo(scores.dtype).min, scores.dtype)
    scores = jnp.where(mask[None, :, :], scores, neg)
    attn = jax.nn.softmax(scores, axis=-1)
    out = jnp.einsum("hqk,kd->hqd", attn, v)             # [hpg, Q, hd]
    out = out.transpose(1, 0, 2).reshape(Q, GQ)          # [Q, 256]
    return out @ WoT_g                                   # [Q, DIM] partial


def kernel(query, key, value, attn_mask, Wq, bq, Wk, bk, Wv, bv,
           q_norm_w, q_norm_b, k_norm_w, k_norm_b, Wo, bo):
    query = np.asarray(query, np.float32)
    key = np.asarray(key, np.float32)
    value = np.asarray(value, np.float32)
    attn_mask = np.asarray(attn_mask, bool)
    Wq = np.asarray(Wq, np.float32); bq = np.asarray(bq, np.float32)
    Wk = np.asarray(Wk, np.float32); bk = np.asarray(bk, np.float32)
    Wv = np.asarray(Wv, np.float32); bv = np.asarray(bv, np.float32)
    Wo = np.asarray(Wo, np.float32); bo = np.asarray(bo, np.float32)

    # device i -> (b = i // G, g = i % G)
    bs = [i // NUM_GROUPS for i in range(NDEV)]
    gs = [i % NUM_GROUPS for i in range(NDEV)]

    def stack(fn):
        return np.stack([fn(b, g) for b, g in zip(bs, gs)])

    q_sh = stack(lambda b, g: query[b])
    k_sh = stack(lambda b, g: key[b])
    v_sh = stack(lambda b, g: value[b])
    m_sh = stack(lambda b, g: attn_mask[b])
    Wq_sh = stack(lambda b, g: Wq[g * GQ:(g + 1) * GQ])
    bq_sh = stack(lambda b, g: bq[g * GQ:(g + 1) * GQ])
    Wk_sh = stack(lambda b, g: Wk[g * HEAD_DIM:(g + 1) * HEAD_DIM])
    bk_sh = stack(lambda b, g: bk[g * HEAD_DIM:(g + 1) * HEAD_DIM])
    Wv_sh = stack(lambda b, g: Wv[g * HEAD_DIM:(g + 1) * HEAD_DIM])
    bv_sh = stack(lambda b, g: bv[g * HEAD_DIM:(g + 1) * HEAD_DIM])
    qnw_sh = stack(lambda b, g: np.asarray(q_norm_w, np.float32))
    qnb_sh = stack(lambda b, g: np.asarray(q_norm_b, np.float32))
    knw_sh = stack(lambda b, g: np.asarray(k_norm_w, np.float32))
    knb_sh = stack(lambda b, g: np.asarray(k_norm_b, np.float32))
    # row-sharded out_proj: partial = out_local @ Wo[:, g-slice].T
    WoT_sh = stack(lambda b, g: Wo[:, g * GQ:(g + 1) * GQ].T.copy())

    part = _gqa_shard(q_sh, k_sh, v_sh, m_sh, Wq_sh, bq_sh, Wk_sh, bk_sh,
                      Wv_sh, bv_sh, qnw_sh, qnb_sh, knw_sh, knb_sh, WoT_sh)
    part = np.asarray(part)                                  # [8, Q, DIM]
    out = part.reshape(B, NUM_GROUPS, part.shape[1], DIM).sum(axis=1) + bo
    return out.astype(np.float32)


# revision 3
# speedup vs baseline: 1.2302x; 1.2302x over previous
"""GroupedQueryAttention kernel for 8 TRN2 NeuronCores.

Sharding: the 8 (batch, kv-group) pairs map 1:1 onto the 8 cores
(B=2 x G=4). Each core holds its group's K/V projection rows, the
matching 4-query-head slice of Wq, and the matching 256-column slice
of Wo (row-sharded out_proj). Each core produces a partial
[Q, DIM] out-proj contribution; the 4-way group reduction + bias is
done on host (cheap: 2*4*2048*1024 adds).
"""
import numpy as np
import jax
import jax.numpy as jnp
from functools import partial

DIM = 1024
NUM_HEADS = 16
NUM_GROUPS = 4
HEAD_DIM = DIM // NUM_HEADS          # 64
HPG = NUM_HEADS // NUM_GROUPS        # 4
GQ = HPG * HEAD_DIM                  # 256: query-proj rows per group
B = 2
SCALE = 1.0 / np.sqrt(HEAD_DIM)
LN_EPS = 1e-5
NDEV = 8


def _ln(x, w, b):
    m = jnp.mean(x, axis=-1, keepdims=True)
    v = jnp.mean(jnp.square(x - m), axis=-1, keepdims=True)
    return (x - m) * jax.lax.rsqrt(v + LN_EPS) * w + b


def _gqa_body(q_in, k_in, v_in, mask, Wq_g, bq_g, Wk_g, bk_g, Wv_g, bv_g,
              qn_w, qn_b, kn_w, kn_b, WoT_g, use_mask):
    # q_in/k_in/v_in: [Q, DIM]; mask: [Q, KV] bool
    q = q_in @ Wq_g.T + bq_g                         # [Q, 256]
    k = k_in @ Wk_g.T + bk_g                         # [KV, 64]
    v = v_in @ Wv_g.T + bv_g                         # [KV, 64]
    Q = q.shape[0]
    q = q.reshape(Q, HPG, HEAD_DIM).transpose(1, 0, 2)   # [hpg, Q, hd]
    q = _ln(q, qn_w, qn_b)
    k = _ln(k, kn_w, kn_b)                               # [KV, hd]
    qb = (q * SCALE).astype(jnp.bfloat16)
    kb = k.astype(jnp.bfloat16)
    vb = v.astype(jnp.bfloat16)
    scores = jnp.einsum("hqd,kd->hqk", qb, kb,
                        preferred_element_type=jnp.float32)  # [hpg, Q, KV]
    if use_mask:
        neg = jnp.asarray(jnp.finfo(scores.dtype).min, scores.dtype)
        scores = jnp.where(mask[None, :, :], scores, neg)
    attn = jax.nn.softmax(scores, axis=-1).astype(jnp.bfloat16)
    out = jnp.einsum("hqk,kd->hqd", attn, vb,
                     preferred_element_type=jnp.float32)     # [hpg, Q, hd]
    out = out.transpose(1, 0, 2).reshape(Q, GQ)          # [Q, 256]
    return out @ WoT_g                                   # [Q, DIM] partial


_gqa_shard = jax.pmap(partial(_gqa_body, use_mask=True), axis_name="x")
_gqa_shard_nomask = jax.pmap(partial(_gqa_body, use_mask=False), axis_name="x")


def kernel(query, key, value, attn_mask, Wq, bq, Wk, bk, Wv, bv,
           q_norm_w, q_norm_b, k_norm_w, k_norm_b, Wo, bo):
    query = np.asarray(query, np.float32)
    key = np.asarray(key, np.float32)
    value = np.asarray(value, np.float32)
    attn_mask = np.asarray(attn_mask, bool)
    Wq = np.asarray(Wq, np.float32); bq = np.asarray(bq, np.float32)
    Wk = np.asarray(Wk, np.float32); bk = np.asarray(bk, np.float32)
    Wv = np.asarray(Wv, np.float32); bv = np.asarray(bv, np.float32)
    Wo = np.asarray(Wo, np.float32); bo = np.asarray(bo, np.float32)

    # device i -> (b = i // G, g = i % G)
    bs = [i // NUM_GROUPS for i in range(NDEV)]
    gs = [i % NUM_GROUPS for i in range(NDEV)]

    def stack(fn):
        return np.stack([fn(b, g) for b, g in zip(bs, gs)])

    q_sh = stack(lambda b, g: query[b])
    k_sh = stack(lambda b, g: key[b])
    v_sh = stack(lambda b, g: value[b])
    m_sh = stack(lambda b, g: attn_mask[b])
    Wq_sh = stack(lambda b, g: Wq[g * GQ:(g + 1) * GQ])
    bq_sh = stack(lambda b, g: bq[g * GQ:(g + 1) * GQ])
    Wk_sh = stack(lambda b, g: Wk[g * HEAD_DIM:(g + 1) * HEAD_DIM])
    bk_sh = stack(lambda b, g: bk[g * HEAD_DIM:(g + 1) * HEAD_DIM])
    Wv_sh = stack(lambda b, g: Wv[g * HEAD_DIM:(g + 1) * HEAD_DIM])
    bv_sh = stack(lambda b, g: bv[g * HEAD_DIM:(g + 1) * HEAD_DIM])
    qnw_sh = stack(lambda b, g: np.asarray(q_norm_w, np.float32))
    qnb_sh = stack(lambda b, g: np.asarray(q_norm_b, np.float32))
    knw_sh = stack(lambda b, g: np.asarray(k_norm_w, np.float32))
    knb_sh = stack(lambda b, g: np.asarray(k_norm_b, np.float32))
    # row-sharded out_proj: partial = out_local @ Wo[:, g-slice].T
    WoT_sh = stack(lambda b, g: Wo[:, g * GQ:(g + 1) * GQ].T.copy())

    fn = _gqa_shard_nomask if attn_mask.all() else _gqa_shard
    part = fn(q_sh, k_sh, v_sh, m_sh, Wq_sh, bq_sh, Wk_sh, bk_sh,
              Wv_sh, bv_sh, qnw_sh, qnb_sh, knw_sh, knb_sh, WoT_sh)
    part = np.asarray(part)                                  # [8, Q, DIM]
    out = part.reshape(B, NUM_GROUPS, part.shape[1], DIM).sum(axis=1) + bo
    return out.astype(np.float32)
